# revision 10
# baseline (speedup 1.0000x reference)
"""VGG16+STN Trainium2 kernel — 8 NeuronCores, 2 images x 4-way spatial shard.

Sharding: core = img*4 + part. Each core computes a 4-row-split of its image's
VGG pipeline with halo AllGathers at pool boundaries, then the STN head
(replicated within each 4-core group) on the full 16x16 feature map.

Numerics: bf16 matmul inputs, fp32 PSUM accumulation, bf16 activations in HBM.
The grid-sample is computed as base-conv + small-delta correction (dG = G -
onehot has all entries O(theta-identity), so bf16 noise on it is 2nd order);
the theta path stays fp32 on device.
"""
import numpy as np
import ml_dtypes

BF = ml_dtypes.bfloat16
DEBUG = False

VGG_CFG = [(3, 64), (64, 64), (64, 128), (128, 128), (128, 256), (256, 256),
           (256, 256), (256, 512), (512, 512), (512, 512), (512, 512),
           (512, 512), (512, 512)]

# stages: (conv indices, W at input res, own rows per core, halo h)
STAGES = [
    ([0, 1], 512, 128, 2),
    ([2, 3], 256, 64, 2),
    ([4, 5, 6], 128, 32, 3),
    ([7, 8, 9], 64, 16, 3),
    ([10, 11, 12], 32, 8, 3),
]
FMAP = 16
NCLS = 21

_CACHE = {}


# ---------------------------------------------------------------- host prep
def _prep_weights(params):
    t = {}
    vgg = [(np.asarray(w, np.float32), np.asarray(b, np.float32)) for w, b in params['vgg']]
    loc = [(np.asarray(w, np.float32), np.asarray(b, np.float32)) for w, b in params['loc']]
    (w1, b1), (w2, b2) = params['fc']
    w1 = np.asarray(w1, np.float32); w2 = np.asarray(w2, np.float32)
    b2 = np.asarray(b2, np.float32)
    ws, bs = params['stn']
    ws = np.asarray(ws, np.float32); bs = np.asarray(bs, np.float32)

    def chunk_layout(w):
        # w [co, ci, 3, 3] -> [128, cc, 9, co]: arr[p, c, t, o] = w[o, c*128+p, ky, kx]
        co, ci = w.shape[0], w.shape[1]
        cc = ci // 128
        a = w.reshape(co, cc, 128, 9)
        return np.ascontiguousarray(a.transpose(2, 1, 3, 0)).astype(BF)

    def dup2_layout(w):
        co = w.shape[0]
        a = np.zeros((128, 6, co), np.float32)
        for kx in range(3):
            a[0:64, kx] = w[:, :, 0, kx].T
            a[64:128, kx] = w[:, :, 1, kx].T
            a[64:128, 3 + kx] = w[:, :, 2, kx].T
        return a.astype(BF)

    w0 = vgg[0][0]
    a0 = np.zeros((27, 64), np.float32)
    for t9 in range(9):
        ky, kx = divmod(t9, 3)
        a0[3 * t9:3 * t9 + 3] = w0[:, :, ky, kx].T
    t['w0'] = a0.astype(BF)
    t['w1'] = dup2_layout(vgg[1][0])
    t['w2'] = dup2_layout(vgg[2][0])
    for i in range(3, 13):
        t[f'w{i}'] = chunk_layout(vgg[i][0])
    t['wloc1'] = chunk_layout(loc[0][0])
    t['wloc2'] = chunk_layout(loc[1][0])
    t['wstn'] = chunk_layout(ws)
    t['bstn'] = bs.reshape(NCLS, 1).astype(np.float32)
    t['wfc1'] = np.ascontiguousarray(
        w1.reshape(64, 4, 128).transpose(2, 1, 0).reshape(128, 256)).astype(BF)
    t['wfc2'] = np.ascontiguousarray(w2.T).astype(BF)  # [64, 6]
    t['bfc2'] = b2.reshape(6, 1).astype(np.float32)
    t['clipb'] = np.array([[1e30], [1e-4], [1e30], [1e-4], [1e30], [1e30]], np.float32)

    lin = np.linspace(-1, 1, 18, dtype=np.float32)
    ii, jj = np.meshgrid(np.arange(16), np.arange(16), indexing='ij')
    p_i, p_j = ii.ravel(), jj.ravel()
    uu, vv = np.meshgrid(np.arange(3), np.arange(3), indexing='ij')
    k_u, k_v = uu.ravel(), vv.ravel()
    t['pos_gx'] = lin[p_j[:, None] + k_v[None, :]].astype(np.float32)  # [256, 9]
    t['pos_gy'] = lin[p_i[:, None] + k_u[None, :]].astype(np.float32)
    t['iota18'] = np.broadcast_to(np.arange(18, dtype=np.float32), (128, 18)).copy()
    # onehot [256, 9, 18, 18] (k, r, c): 1 at r=i+u, c=j+v
    oh = np.zeros((256, 9, 18, 18), np.float32)
    for p in range(256):
        for k in range(9):
            oh[p, k, p_i[p] + k_u[k], p_j[p] + k_v[k]] = 1.0
    t['onehot'] = oh.astype(BF)
    t['identbf'] = np.eye(128, dtype=np.float32).astype(BF)
    t['identf32'] = np.eye(128, dtype=np.float32)
    return t


def _prep_core_inputs(x, weights):
    maps = []
    for core in range(8):
        img, part = divmod(core, 4)
        g0 = part * 128
        xin = np.zeros((3, 132, 514), np.float32)
        lo, hi = g0 - 2, g0 + 130
        clo, chi = max(lo, 0), min(hi, 512)
        xin[:, clo - lo:chi - lo, 1:513] = x[img, :, clo:chi, :]
        m = np.zeros((128, 8), np.float32)
        if part > 0:
            m[:, part - 1] = 1.0        # top halo <- slot (part-1)'s bottom strip
        if part < 3:
            m[:, 4 + part + 1] = 1.0    # bottom halo <- slot (part+1)'s top strip
        d = {'xin': xin.astype(BF), 'masks': m}
        d.update(weights)
        maps.append(d)
    return maps


# ---------------------------------------------------------------- bass build
def _build():
    import concourse.bass as bass
    import concourse.tile as tile
    from concourse import bacc, mybir
    from contextlib import ExitStack

    dt = mybir.dt
    AF = mybir.ActivationFunctionType
    ALU = mybir.AluOpType
    GROUPS = [[0, 1, 2, 3], [4, 5, 6, 7]]

    nc = bacc.Bacc("TRN2", target_bir_lowering=False, debug=False, num_devices=8)

    def din(name, shape, dty=dt.bfloat16):
        return nc.dram_tensor(name, list(shape), dty, kind="ExternalInput")

    xin = din('xin', [3, 132, 514])
    masks = din('masks', [128, 8], dt.float32)
    wt = {}
    wt['w0'] = din('w0', [27, 64])
    wt['w1'] = din('w1', [128, 6, 64])
    wt['w2'] = din('w2', [128, 6, 128])
    for i in range(3, 13):
        ci, co = VGG_CFG[i]
        wt[f'w{i}'] = din(f'w{i}', [128, ci // 128, 9, co])
    wt['wloc1'] = din('wloc1', [128, 4, 9, 512])
    wt['wloc2'] = din('wloc2', [128, 4, 9, 512])
    wt['wstn'] = din('wstn', [128, 4, 9, NCLS])
    wt['wfc1'] = din('wfc1', [128, 256])
    wt['wfc2'] = din('wfc2', [64, 6])
    bstn = din('bstn', [NCLS, 1], dt.float32)
    bfc2 = din('bfc2', [6, 1], dt.float32)
    clipb = din('clipb', [6, 1], dt.float32)
    pos_gx = din('pos_gx', [256, 9], dt.float32)
    pos_gy = din('pos_gy', [256, 9], dt.float32)
    iota18 = din('iota18', [128, 18], dt.float32)
    onehot = din('onehot', [256, 9, 18, 18])
    identbf = din('identbf', [128, 128])
    identf32 = din('identf32', [128, 128], dt.float32)

    probs_out = nc.dram_tensor('probs', [NCLS, 256], dt.float32, kind="ExternalOutput")
    theta_out = nc.dram_tensor('theta_dbg', [6, 256], dt.float32, kind="ExternalOutput")

    def dbuf(name, c, r, w2):
        return nc.dram_tensor(name, [c, r, w2], dt.bfloat16)

    a0 = dbuf('a0', 64, 130, 514); a1 = dbuf('a1', 64, 128, 514)
    s2in = dbuf('s2in', 64, 68, 258)
    b0 = dbuf('b0', 128, 66, 258); b1 = dbuf('b1', 128, 64, 258)
    s3in = dbuf('s3in', 128, 38, 130)
    c0 = dbuf('c0', 256, 36, 130); c1 = dbuf('c1', 256, 34, 130); c2 = dbuf('c2', 256, 32, 130)
    s4in = dbuf('s4in', 256, 22, 66)
    d0 = dbuf('d0', 512, 20, 66); d1 = dbuf('d1', 512, 18, 66); d2 = dbuf('d2', 512, 16, 66)
    s5in = dbuf('s5in', 512, 14, 34)
    e0 = dbuf('e0', 512, 12, 34); e1 = dbuf('e1', 512, 10, 34); e2 = dbuf('e2', 512, 8, 34)
    featp_d = dbuf('featp_d', 512, 18, 18)
    locp_d = dbuf('locp_d', 512, 18, 18)
    g2_d = dbuf('g2_d', 512, 16, 18)

    ag = {}
    for si in range(1, 5):
        _, W, own, h = STAGES[si]
        C = VGG_CFG[STAGES[si - 1][0][-1]][1]
        ag[si] = (nc.dram_tensor(f'agin{si}', [2 * h, C, W], dt.bfloat16),
                  nc.dram_tensor(f'agout{si}', [8 * h, C, W], dt.bfloat16))
    ag5_in = nc.dram_tensor('agin5f', [512, 4, 16], dt.bfloat16)
    ag5_out = nc.dram_tensor('agout5f', [2048, 4, 16], dt.bfloat16)

    dbg_taps = {}
    if DEBUG:
        for name, buf in [('a1', a1), ('s2in', s2in), ('s3in', s3in), ('s4in', s4in),
                          ('s5in', s5in), ('featp', featp_d), ('g2', g2_d)]:
            dbg_taps[name] = (nc.dram_tensor(
                f'dbg_{name}', list(buf.shape), dt.bfloat16, kind="ExternalOutput"), buf)

    with ExitStack() as ctx:
        tc = ctx.enter_context(tile.TileContext(nc))
        wpool = ctx.enter_context(tc.tile_pool(name="wpool", bufs=2))
        inpool = ctx.enter_context(tc.tile_pool(name="inpool", bufs=3))
        outpool = ctx.enter_context(tc.tile_pool(name="outpool", bufs=2))
        pspool = ctx.enter_context(tc.tile_pool(name="pspool", bufs=6, space="PSUM"))
        pshead = ctx.enter_context(tc.tile_pool(name="pshead", bufs=2, space="PSUM"))
        misc = ctx.enter_context(tc.tile_pool(name="misc", bufs=1))
        hpool = ctx.enter_context(tc.tile_pool(name="hpool", bufs=1))

        # ---------------- zero side columns / border rows
        zt = misc.tile([128, 144], dt.bfloat16, tag="zeros")
        nc.gpsimd.memset(zt[:], 0.0)
        for buf in [a0, a1, s2in, b0, b1, s3in, c0, c1, c2, s4in, d0, d1, d2,
                    s5in, e0, e1, e2, featp_d, locp_d, g2_d]:
            C, R, W2 = buf.shape
            for cb in range((C + 127) // 128):
                p = min(128, C - cb * 128)
                for col in (0, W2 - 1):
                    nc.sync.dma_start(
                        buf.ap()[cb * 128:cb * 128 + p, :, col:col + 1],
                        zt[0:p, 0:R].unsqueeze(2))
        for buf in (featp_d, locp_d):
            for cb in range(4):
                for row in (0, 17):
                    nc.sync.dma_start(
                        buf.ap()[cb * 128:(cb + 1) * 128, row:row + 1, :],
                        zt[0:128, 0:18].unsqueeze(1))

        mt = misc.tile([128, 8], dt.float32, tag="masks")
        nc.sync.dma_start(mt[:], masks.ap())

        # ---------------- generic conv
        def emit_conv(in_d, out_d, ci, co, W, w_dram, mode, RB, rout, out_r0=0):
            W2 = W + 2
            nr_max = max(1, 512 // W)
            cc = max(1, ci // 128)
            for rb in range(0, rout, RB):
                blk = min(RB, rout - rb)
                if mode == "chunk":
                    it = inpool.tile([128, cc, blk + 2, W2], dt.bfloat16, tag="in")
                    src = in_d.ap().rearrange("(c p) r w -> p c r w", p=128)
                    nc.sync.dma_start(it[:], src[:, :, rb:rb + blk + 2, :])
                elif mode == "dup2":
                    it = inpool.tile([128, blk + 2, W2], dt.bfloat16, tag="in")
                    nc.sync.dma_start(it[0:64, 0:blk + 1], in_d.ap()[:, rb:rb + blk + 1, :])
                    nc.sync.dma_start(it[64:128, 0:blk + 1], in_d.ap()[:, rb + 1:rb + blk + 2, :])
                else:  # dup9
                    it = inpool.tile([27, blk, W], dt.bfloat16, tag="in")
                    for t in range(9):
                        ky, kx = divmod(t, 3)
                        nc.sync.dma_start(it[3 * t:3 * t + 3],
                                          in_d.ap()[:, rb + ky:rb + ky + blk, kx:kx + W])
                for ob in range((co + 127) // 128):
                    cob = min(128, co - ob * 128)
                    if mode == "dup9":
                        w_sb = wpool.tile([27, 64], dt.bfloat16, tag="w")
                        nc.sync.dma_start(w_sb[:], w_dram.ap())
                    elif mode == "dup2":
                        w_sb = wpool.tile([128, 6, cob], dt.bfloat16, tag="w")
                        nc.sync.dma_start(w_sb[:], w_dram.ap()[:, :, ob * 128:ob * 128 + cob])
                    else:
                        w_sb = wpool.tile([128, cc, 9, cob], dt.bfloat16, tag="w")
                        nc.sync.dma_start(w_sb[:], w_dram.ap()[:, :, :, ob * 128:ob * 128 + cob])
                    ot = outpool.tile([cob, blk, W], dt.bfloat16, tag="out")
                    r = 0
                    while r < blk:
                        nr = min(blk - r, nr_max)
                        ps = pspool.tile([cob, nr, W], dt.float32, tag="ps")
                        mms = []
                        if mode == "dup9":
                            mms.append((w_sb[:, :], it[:, r:r + nr, :]))
                        elif mode == "dup2":
                            for kx in range(3):
                                mms.append((w_sb[:, kx, :], it[:, r:r + nr, kx:kx + W]))
                            for kx in range(3):
                                mms.append((w_sb[64:128, 3 + kx, :],
                                            it[64:128, r + 1:r + 1 + nr, kx:kx + W]))
                        else:
                            for c in range(cc):
                                for t in range(9):
                                    ky, kx = divmod(t, 3)
                                    mms.append((w_sb[:, c, t, :],
                                                it[:, c, r + ky:r + ky + nr, kx:kx + W]))
                        for i, (lh, rh) in enumerate(mms):
                            nc.tensor.matmul(ps[:], lh, rh,
                                             start=(i == 0), stop=(i == len(mms) - 1))
                        nc.scalar.activation(ot[:, r:r + nr, :], ps[:], AF.Relu)
                        r += nr
                    nc.sync.dma_start(
                        out_d.ap()[ob * 128:ob * 128 + cob,
                                   out_r0 + rb:out_r0 + rb + blk, 1:W + 1], ot[:])

        # ---------------- maxpool 2x2 (rows [0, R) of in_d -> dst rows [r0, r0+R/2))
        def emit_pool(in_d, dst_d, dst_r0, W, rblk=16):
            C, R, W2 = in_d.shape
            Wh = W // 2
            for cb in range((C + 127) // 128):
                p = min(128, C - cb * 128)
                for r0 in range(0, R, rblk):
                    rr = min(rblk, R - r0)
                    it = inpool.tile([p, rr, W], dt.bfloat16, tag="in")
                    nc.sync.dma_start(it[:], in_d.ap()[cb * 128:cb * 128 + p,
                                                       r0:r0 + rr, 1:W + 1])
                    t1 = outpool.tile([p, rr, Wh], dt.bfloat16, tag="plt")
                    nc.vector.tensor_max(t1[:], it[:, :, 0:W:2], it[:, :, 1:W:2])
                    t2 = outpool.tile([p, rr // 2, Wh], dt.bfloat16, tag="plo")
                    nc.vector.tensor_max(t2[:], t1[:, 0:rr:2, :], t1[:, 1:rr:2, :])
                    nc.sync.dma_start(
                        dst_d.ap()[cb * 128:cb * 128 + p,
                                   dst_r0 + r0 // 2:dst_r0 + (r0 + rr) // 2, 1:Wh + 1], t2[:])

        def emit_pool5(in_d):
            for cb in range(4):
                it = inpool.tile([128, 8, 32], dt.bfloat16, tag="in")
                nc.sync.dma_start(it[:], in_d.ap()[cb * 128:(cb + 1) * 128, :, 1:33])
                t1 = outpool.tile([128, 8, 16], dt.bfloat16, tag="plt")
                nc.vector.tensor_max(t1[:], it[:, :, 0:32:2], it[:, :, 1:32:2])
                t2 = outpool.tile([128, 4, 16], dt.bfloat16, tag="plo")
                nc.vector.tensor_max(t2[:], t1[:, 0:8:2, :], t1[:, 1:8:2, :])
                nc.sync.dma_start(ag5_in.ap()[cb * 128:(cb + 1) * 128], t2[:])

        # ---------------- halo exchange at boundary into stage si's input buffer
        def emit_halo(si, s_in):
            _, W, own, h = STAGES[si]
            C = s_in.shape[0]
            agi, ago = ag[si]
            nc.sync.dma_start(agi.ap()[0:h],
                              s_in.ap()[:, h:2 * h, 1:W + 1].rearrange("c r w -> r c w"))
            nc.sync.dma_start(agi.ap()[h:2 * h],
                              s_in.ap()[:, own:own + h, 1:W + 1].rearrange("c r w -> r c w"))
            nc.gpsimd.collective_compute(
                "AllGather", ALU.bypass, replica_groups=GROUPS,
                ins=[agi.ap().opt()], outs=[ago.ap().opt()])
            cc = (C + 127) // 128
            pd = min(128, C)
            acc = hpool.tile([pd, cc, h, W], dt.bfloat16, tag="hacc")
            tmp = hpool.tile([pd, cc, h, W], dt.bfloat16, tag="htmp")
            for region in range(2):  # 0: top halo <- slots' bottom strips; 1: bottom halo
                strip_sel = 1 - region
                st = hpool.tile([pd, 4, cc, h, W], dt.bfloat16, tag="strips",
                                name=f"strips{si}_{region}")
                for s in range(4):
                    for c in range(cc):
                        src = ago.ap()[s * 2 * h + strip_sel * h:
                                       s * 2 * h + strip_sel * h + h,
                                       c * pd:(c + 1) * pd, :]
                        nc.sync.dma_start(st[:, s, c],
                                          src.rearrange("r p w -> p r w"))
                nc.vector.tensor_scalar_mul(acc[:], st[:, 0],
                                            mt[0:pd, 4 * region:4 * region + 1])
                for s in range(1, 4):
                    nc.vector.tensor_scalar_mul(tmp[:], st[:, s],
                                                mt[0:pd, 4 * region + s:4 * region + s + 1])
                    nc.vector.tensor_add(acc[:], acc[:], tmp[:])
                if region == 0:
                    dst = s_in.ap()[:, 0:h, 1:W + 1]
                else:
                    dst = s_in.ap()[:, h + own:2 * h + own, 1:W + 1]
                for c in range(cc):
                    nc.sync.dma_start(dst[c * pd:(c + 1) * pd], acc[:, c])

        # ================ VGG pipeline ================
        emit_conv(xin, a0, 3, 64, 512, wt['w0'], "dup9", 16, 130)
        emit_conv(a0, a1, 64, 64, 512, wt['w1'], "dup2", 12, 128)
        emit_pool(a1, s2in, 2, 512, rblk=16)
        emit_halo(1, s2in)
        emit_conv(s2in, b0, 64, 128, 256, wt['w2'], "dup2", 12, 66)
        emit_conv(b0, b1, 128, 128, 256, wt['w3'], "chunk", 32, 64)
        emit_pool(b1, s3in, 3, 256, rblk=16)
        emit_halo(2, s3in)
        emit_conv(s3in, c0, 128, 256, 128, wt['w4'], "chunk", 36, 36)
        emit_conv(c0, c1, 256, 256, 128, wt['w5'], "chunk", 34, 34)
        emit_conv(c1, c2, 256, 256, 128, wt['w6'], "chunk", 32, 32)
        emit_pool(c2, s4in, 3, 128, rblk=16)
        emit_halo(3, s4in)
        emit_conv(s4in, d0, 256, 512, 64, wt['w7'], "chunk", 20, 20)
        emit_conv(d0, d1, 512, 512, 64, wt['w8'], "chunk", 18, 18)
        emit_conv(d1, d2, 512, 512, 64, wt['w9'], "chunk", 16, 16)
        emit_pool(d2, s5in, 3, 64, rblk=16)
        emit_halo(4, s5in)
        emit_conv(s5in, e0, 512, 512, 32, wt['w10'], "chunk", 12, 12)
        emit_conv(e0, e1, 512, 512, 32, wt['w11'], "chunk", 10, 10)
        emit_conv(e1, e2, 512, 512, 32, wt['w12'], "chunk", 8, 8)
        emit_pool5(e2)

        # feature AllGather -> assemble featp_d [512, 18, 18]
        nc.gpsimd.collective_compute(
            "AllGather", ALU.bypass, replica_groups=GROUPS,
            ins=[ag5_in.ap().opt()], outs=[ag5_out.ap().opt()])
        for slot in range(4):
            ft = inpool.tile([128, 4, 4, 16], dt.bfloat16, tag="in")
            src = ag5_out.ap()[slot * 512:(slot + 1) * 512].rearrange(
                "(c p) r w -> p c r w", p=128)
            nc.sync.dma_start(ft[:], src)
            for c in range(4):
                nc.sync.dma_start(
                    featp_d.ap()[c * 128:(c + 1) * 128,
                                 1 + 4 * slot:1 + 4 * slot + 4, 1:17], ft[:, c])

        # ================ head ================
        emit_conv(featp_d, locp_d, 512, 512, 16, wt['wloc1'], "chunk", 16, 16, out_r0=1)
        emit_conv(locp_d, g2_d, 512, 512, 16, wt['wloc2'], "chunk", 16, 16)

        g2 = hpool.tile([128, 4, 16, 16], dt.bfloat16, tag="g2")
        for c in range(4):
            nc.sync.dma_start(g2[:, c], g2_d.ap()[c * 128:(c + 1) * 128, :, 1:17])

        # fc1
        wf1 = misc.tile([128, 4, 64], dt.bfloat16, tag="wf1")
        nc.sync.dma_start(wf1[:], wt['wfc1'].ap().rearrange("p (c o) -> p c o", c=4))
        ps1 = pshead.tile([64, 256], dt.float32, tag="psh")
        for c in range(4):
            nc.tensor.matmul(ps1[:], wf1[:, c, :], g2[:, c],
                             start=(c == 0), stop=(c == 3))
        t1 = hpool.tile([64, 256], dt.bfloat16, tag="t1")
        nc.scalar.activation(t1[:], ps1[:], AF.Relu)

        # fc2 + b2 -> theta [6, 256] fp32; clip rows 1,3
        wf2 = misc.tile([64, 6], dt.bfloat16, tag="wf2")
        nc.sync.dma_start(wf2[:], wt['wfc2'].ap())
        ps2 = pshead.tile([6, 256], dt.float32, tag="psh")
        nc.tensor.matmul(ps2[:], wf2[:], t1[:], start=True, stop=True)
        bf2 = misc.tile([6, 1], dt.float32, tag="bf2")
        nc.sync.dma_start(bf2[:], bfc2.ap())
        th = hpool.tile([6, 256], dt.float32, tag="theta")
        nc.vector.tensor_scalar_add(th[:], ps2[:], bf2[:])
        cb_t = misc.tile([6, 1], dt.float32, tag="clipb")
        nc.sync.dma_start(cb_t[:], clipb.ap())
        ncb_t = misc.tile([6, 1], dt.float32, tag="nclipb")
        nc.vector.tensor_scalar_mul(ncb_t[:], cb_t[:], -1.0)
        th2 = hpool.tile([6, 256], dt.float32, tag="theta2")
        nc.vector.tensor_scalar(th2[:], th[:], cb_t[:], None, ALU.min)
        nc.vector.tensor_scalar(th[:], th2[:], ncb_t[:], None, ALU.max)
        nc.sync.dma_start(theta_out.ap(), th[:])

        # transpose theta -> thT[pt] [128, 6] fp32
        idf = misc.tile([128, 128], dt.float32, tag="idf")
        nc.sync.dma_start(idf[:], identf32.ap())
        idb = misc.tile([128, 128], dt.bfloat16, tag="idb")
        nc.sync.dma_start(idb[:], identbf.ap())
        thT = [hpool.tile([128, 6], dt.float32, tag=f"thT{i}", name=f"thT{i}") for i in range(2)]
        for i in range(2):
            pst = pshead.tile([128, 6], dt.float32, tag="psh")
            nc.tensor.transpose(pst[:], th[:, i * 128:(i + 1) * 128], idf[0:6, 0:6])
            nc.vector.tensor_copy(thT[i][:], pst[:])

        gxs = misc.tile([128, 2, 9], dt.float32, tag="gxs")
        gys = misc.tile([128, 2, 9], dt.float32, tag="gys")
        nc.sync.dma_start(gxs[:], pos_gx.ap().rearrange("(t p) k -> p t k", p=128))
        nc.sync.dma_start(gys[:], pos_gy.ap().rearrange("(t p) k -> p t k", p=128))
        io18 = misc.tile([128, 18], dt.float32, tag="io18")
        nc.sync.dma_start(io18[:], iota18.ap())

        # Delta-G per ptile -> dG_sb[schunk] [108, 9, 2, 128] bf16 (q = k*256 + pt*128 + p)
        dG_sb = [hpool.tile([108, 9, 2, 128], dt.bfloat16, tag=f"dG{s}", name=f"dG{s}") for s in range(3)]
        for pt in range(2):
            sx = hpool.tile([128, 9], dt.float32, tag="sx")
            sy = hpool.tile([128, 9], dt.float32, tag="sy")
            tmp9 = hpool.tile([128, 9], dt.float32, tag="tmp9")
            nc.vector.tensor_scalar_mul(sx[:], gxs[:, pt], thT[pt][:, 0:1])
            nc.vector.tensor_scalar_mul(tmp9[:], gys[:, pt], thT[pt][:, 1:2])
            nc.vector.tensor_add(sx[:], sx[:], tmp9[:])
            nc.vector.tensor_scalar_add(sx[:], sx[:], thT[pt][:, 2:3])
            nc.vector.tensor_scalar_mul(sy[:], gxs[:, pt], thT[pt][:, 3:4])
            nc.vector.tensor_scalar_mul(tmp9[:], gys[:, pt], thT[pt][:, 4:5])
            nc.vector.tensor_add(sy[:], sy[:], tmp9[:])
            nc.vector.tensor_scalar_add(sy[:], sy[:], thT[pt][:, 5:6])
            ix = hpool.tile([128, 9], dt.float32, tag="ix")
            iy = hpool.tile([128, 9], dt.float32, tag="iy")
            nc.scalar.activation(ix[:], sx[:], AF.Copy, bias=8.5, scale=8.5)
            nc.scalar.activation(iy[:], sy[:], AF.Copy, bias=8.5, scale=8.5)
            wx = hpool.tile([128, 9, 18], dt.float32, tag="wx")
            wyt = hpool.tile([128, 9, 18], dt.float32, tag="wy")
            for k in range(9):
                # |c - ix| == |ix - c|
                nc.vector.tensor_scalar(wx[:, k], io18[:], ix[:, k:k + 1], None, ALU.subtract)
                nc.vector.tensor_scalar(wyt[:, k], io18[:], iy[:, k:k + 1], None, ALU.subtract)
            nc.scalar.activation(wx[:], wx[:], AF.Abs)
            nc.scalar.activation(wyt[:], wyt[:], AF.Abs)
            nc.scalar.activation(wx[:], wx[:], AF.Relu, bias=1.0, scale=-1.0)
            nc.scalar.activation(wyt[:], wyt[:], AF.Relu, bias=1.0, scale=-1.0)
            # GA[p; k, r, c] = wy[p,k,r] * wx[p,k,c]
            GA = hpool.tile([128, 9, 18, 18], dt.float32, tag="GA")
            for r in range(18):
                nc.vector.tensor_mul(GA[:, :, r, :], wx[:, :, :],
                                     wyt[:, :, r:r + 1].broadcast_to((128, 9, 18)))
            oh_t = hpool.tile([128, 9, 18, 18], dt.bfloat16, tag="oh")
            nc.sync.dma_start(oh_t[:],
                              onehot.ap()[pt * 128:(pt + 1) * 128])
            GAb = hpool.tile([128, 9, 18, 18], dt.bfloat16, tag="GAb")
            nc.vector.tensor_sub(GAb[:], GA[:], oh_t[:])
            for k in range(9):
                flat = GAb[:, k].rearrange("p a b -> p (a b)")
                for s in range(3):
                    pst = pshead.tile([108, 128], dt.bfloat16, tag="psh")
                    nc.tensor.transpose(pst[:], flat[:, s * 108:(s + 1) * 108], idb[:])
                    nc.vector.tensor_copy(dG_sb[s][:, k, pt], pst[:])

        # feat18T chunks [108, 512]
        featp = hpool.tile([128, 4, 18, 18], dt.bfloat16, tag="featp")
        nc.sync.dma_start(featp[:], featp_d.ap().rearrange("(c p) r w -> p c r w", p=128))
        f18T = [hpool.tile([108, 512], dt.bfloat16, tag=f"f18T{s}", name=f"f18T{s}") for s in range(3)]
        for cbk in range(4):
            flat = featp[:, cbk].rearrange("p a b -> p (a b)")
            for s in range(3):
                pst = pshead.tile([108, 128], dt.bfloat16, tag="psh")
                nc.tensor.transpose(pst[:], flat[:, s * 108:(s + 1) * 108], idb[:])
                nc.vector.tensor_copy(f18T[s][:, cbk * 128:(cbk + 1) * 128], pst[:])

        # conv_stn logits = base conv + per-chunk delta
        wst = misc.tile([128, 4, 9, NCLS], dt.bfloat16, tag="wst")
        nc.sync.dma_start(wst[:], wt['wstn'].ap())
        psl = pshead.tile([NCLS, 256], dt.float32, tag="psh")
        for cbk in range(4):
            for t in range(9):
                ky, kx = divmod(t, 3)
                nc.tensor.matmul(psl[:], wst[:, cbk, t],
                                 featp[:, cbk, ky:ky + 16, kx:kx + 16],
                                 start=(cbk == 0 and t == 0), stop=False)
        for cbk in range(4):
            dsampc = hpool.tile([128, 9, 256], dt.bfloat16, tag="dsamp",
                                name=f"dsamp{cbk}")
            q = 0
            while q < 2304:
                nq = min(512, 2304 - q)
                psq = pspool.tile([128, nq], dt.float32, tag="ps")
                for s in range(3):
                    rhs = dG_sb[s].rearrange("s k t p -> s (k t p)")[:, q:q + nq]
                    nc.tensor.matmul(psq[:], f18T[s][:, cbk * 128:(cbk + 1) * 128],
                                     rhs, start=(s == 0), stop=(s == 2))
                nc.vector.tensor_copy(
                    dsampc[:].rearrange("p k q -> p (k q)")[:, q:q + nq], psq[:])
                q += nq
            for t in range(9):
                nc.tensor.matmul(psl[:], wst[:, cbk, t], dsampc[:, t],
                                 start=False, stop=(cbk == 3 and t == 8))
        bst = misc.tile([NCLS, 1], dt.float32, tag="bst")
        nc.sync.dma_start(bst[:], bstn.ap())
        logits = hpool.tile([NCLS, 256], dt.float32, tag="logits")
        nc.vector.tensor_scalar_add(logits[:], psl[:], bst[:])
        ex = hpool.tile([NCLS, 256], dt.float32, tag="ex")
        rowsum = hpool.tile([NCLS, 1], dt.float32, tag="rowsum")
        nc.scalar.activation(ex[:], logits[:], AF.Exp, accum_out=rowsum[:])
        tot = hpool.tile([1, 1], dt.float32, tag="tot")
        nc.gpsimd.tensor_reduce(tot[:], rowsum[:], mybir.AxisListType.XYZWC, ALU.add)
        rinv = hpool.tile([1, 1], dt.float32, tag="rinv")
        nc.vector.reciprocal(rinv[:], tot[:])
        rrow = hpool.tile([1, NCLS], dt.float32, tag="rrow")
        nc.vector.tensor_copy(rrow[:], rinv[:].broadcast_to((1, NCLS)))
        psb = pshead.tile([NCLS, 1], dt.float32, tag="psh")
        nc.tensor.transpose(psb[:], rrow[:], idf[0:1, 0:1])
        rcol = hpool.tile([NCLS, 1], dt.float32, tag="rcol")
        nc.vector.tensor_copy(rcol[:], psb[:])
        probs_t = hpool.tile([NCLS, 256], dt.float32, tag="probs")
        nc.vector.tensor_scalar_mul(probs_t[:], ex[:], rcol[:])
        nc.sync.dma_start(probs_out.ap(), probs_t[:])

        for name, (out_t, buf) in dbg_taps.items():
            nc.sync.dma_start(out_t.ap(), buf.ap())

    nc.compile()
    return nc


# ---------------------------------------------------------------- entry point
def kernel(x, params):
    import time
    from concourse.bass_utils import run_bass_kernel_spmd

    x = np.asarray(x, np.float32)
    if 'nc' not in _CACHE:
        _CACHE['nc'] = _build()
    nc = _CACHE['nc']
    weights = _prep_weights(params)
    in_maps = _prep_core_inputs(x, weights)
    t0 = time.time()
    res = run_bass_kernel_spmd(nc, in_maps, core_ids=list(range(8)))
    kernel.last_run_s = time.time() - t0
    kernel.last_results = res.results
    probs = np.stack([res.results[0]['probs'].reshape(NCLS, FMAP, FMAP),
                      res.results[4]['probs'].reshape(NCLS, FMAP, FMAP)])
    scores = probs.sum((2, 3))
    return scores.astype(np.float32), probs.astype(np.float32)
